# revision 1
# baseline (speedup 1.0000x reference)
"""Bidirectional chamfer loss kernel for Trainium2 (8 NeuronCores).

Problem (hardcoded): B=2 batches, V1=8192 gt points, V2=8192 pred points, 3D.
  d2[b,i,j] = max(0, |xp_i|^2 + |gt_j|^2 - 2 xp_i.gt_j),  xp = x_pred * mask
  loss_pred2gt[b,i] = sqrt(min_j d2) * 100
  loss_gt2pred[b,j] = sqrt(min_i d2) * 100
  loss_conf = (loss_pred2gt * conf - ln(conf)) * mask ; loss_pred2gt *= mask

Sharding: 8 cores = 2 batches x 4 V2-slices (2048 preds/core vs full 8192 gt).
Each core computes its pred2gt slice exactly, and a partial gt2pred
(min over its 2048 preds); the host combines partials with np.minimum
(sqrt is monotone, so combining after sqrt*100 is exact).

Device kernel (per core, SPMD):
  One K=5 matmul per (pred-tile 128, gt-chunk 512) produces d2 directly in
  PSUM:  A rows [-2 xp_x, -2 xp_y, -2 xp_z, |xp|^2, 1]
         G rows [gt_x,    gt_y,    gt_z,    1,      |gt|^2]
  (host assembles these augmented operands -- pure input layout; all of the
  O(V2*V1) distance/min work runs on device).
  DVE tensor_tensor(min) folds each PSUM tile into a per-pred-tile row
  accumulator and a per-gt-chunk column accumulator; rows finish with a
  free-dim reduce_min, columns with PE 128x128 transposes + reduce_min.

Sync-wait discipline: the TPB ISA allows ONE semaphore wait per
instruction and Tile does not legalize beyond that, so the kernel is
structured so every instruction has at most one cross-engine dependency
not already covered by that engine's earlier waits: accumulator init on
the DVE itself, one fused input DMA per consumer chain, and the transpose
identity laundered through a DVE copy so transposes depend only on the
DVE clock.
"""

import numpy as np

B = 2
V1 = 8192  # gt points
V2 = 8192  # pred points (total)
N_CORES = 8
SLICES = N_CORES // B  # V2-slices per batch
V2C = V2 // SLICES  # pred points per core

_BUILT = {}


def _build(v1, v2c, mm_dtype_name="float32", repeat=1):
    import concourse.tile as tile
    from concourse import bacc, mybir

    f32 = mybir.dt.float32
    mm_dt = getattr(mybir.dt, mm_dtype_name)
    MIN = mybir.AluOpType.min
    MUL = mybir.AluOpType.mult
    SUB = mybir.AluOpType.subtract
    X = mybir.AxisListType.X
    AF = mybir.ActivationFunctionType

    npt = v2c // 128  # pred tiles
    ngc = v1 // 512  # gt chunks (matmul moving dim)
    ngt = v1 // 128  # gt output tiles
    BIG = 3.0e38

    # Bacc (not raw Bass): its compile() legalizes the TRN2 one-wait-per-
    # instruction constraint by splitting sync waits into event semaphores
    nc = bacc.Bacc()
    ag_in = nc.dram_tensor("ag", [5, v2c + v1], mm_dt, kind="ExternalInput")
    mc_in = nc.dram_tensor("mc", [128, 2 * npt], f32, kind="ExternalInput")
    # one fused output -> one DMA queue -> fewer kernel-tail drain waits
    o_all = nc.dram_tensor("o_all", [128, 2 * npt + ngt], f32, kind="ExternalOutput")

    with tile.TileContext(nc) as tc:
        with (
            tc.tile_pool(name="persist", bufs=1) as P,
            tc.tile_pool(name="rowp", bufs=2) as RP,
            tc.tile_pool(name="small", bufs=1) as SP,
            tc.tile_pool(name="mmps", bufs=6, space="PSUM") as MMPS,
            tc.tile_pool(name="trps", bufs=2, space="PSUM") as TRPS,
        ):
            AG = P.tile([5, v2c + v1], mm_dt, tag="AG")
            A = AG[:, 0:v2c]
            G = AG[:, v2c : v2c + v1]
            MC = P.tile([128, 2 * npt], f32, tag="MC")
            mc_sb = P.tile([128, 2 * npt], f32, tag="mc_sb")
            mask_ep = mc_sb[:, 0:npt]
            conf_ep = mc_sb[:, npt : 2 * npt]
            ident_pool = P.tile([128, 128], f32, tag="identp")
            ident = P.tile([128, 128], f32, tag="ident")
            colacc = [
                P.tile([128, 512], f32, tag=f"col{g}", name=f"col{g}")
                for g in range(ngc)
            ]
            p2g_min = P.tile([128, npt], f32, tag="p2gmin")
            g2p_min = P.tile([128, ngt], f32, tag="g2pmin")

            # identity for PE transpose, built on gpsimd then laundered
            # through a DVE copy so its consumers sit in the DVE clock domain
            nc.gpsimd.memset(ident_pool[:], 0.0)
            nc.gpsimd.affine_select(
                out=ident_pool[:],
                in_=ident_pool[:],
                compare_op=mybir.AluOpType.not_equal,
                fill=1.0,
                base=0,
                pattern=[[-1, 128]],
                channel_multiplier=1,
            )
            nc.vector.tensor_copy(ident[:], ident_pool[:])

            # ---- input staging (single DMA per operand) ----
            nc.sync.dma_start(AG[:], ag_in[:, :])
            nc.sync.dma_start(MC[:], mc_in[:, :])
            # pull MC into the DVE's clock domain once; epilogue then has
            # no direct DMA dependencies
            nc.vector.tensor_copy(mc_sb[:], MC[:])

            # accumulator init on the DVE itself (no cross-engine sems)
            for g in range(ngc):
                nc.vector.memset(colacc[g][:], BIG)

            # ---- main loop: one matmul + two DVE min-folds per tile ----
            # repeat>1 re-runs the whole loop (idempotent min-folds) for
            # work-scaling timing experiments
            for pt in [p for _ in range(repeat) for p in range(npt)]:
                rowacc = RP.tile([128, 512], f32, tag="rowacc")
                lhsT = A[:, pt * 128 : (pt + 1) * 128]
                for gc in range(ngc):
                    ps = MMPS.tile([128, 512], f32, tag="mm")
                    nc.tensor.matmul(
                        ps[:],
                        lhsT,
                        G[:, gc * 512 : (gc + 1) * 512],
                        start=True,
                        stop=True,
                    )
                    if gc == 0:
                        nc.vector.tensor_copy(rowacc[:], ps[:])
                    else:
                        nc.vector.tensor_tensor(rowacc[:], rowacc[:], ps[:], op=MIN)
                    nc.vector.tensor_tensor(
                        colacc[gc][:], colacc[gc][:], ps[:], op=MIN
                    )
                nc.vector.tensor_reduce(
                    p2g_min[:, pt : pt + 1], rowacc[:], axis=X, op=MIN
                )

            # ---- column (gt2pred) finish: transpose 128x128 blocks + reduce ----
            for gc in range(ngc):
                for q in range(4):
                    tp = TRPS.tile([128, 128], f32, tag="tr")
                    nc.tensor.transpose(
                        tp[:], colacc[gc][:, q * 128 : (q + 1) * 128], ident[:]
                    )
                    j = gc * 4 + q
                    nc.vector.tensor_reduce(
                        g2p_min[:, j : j + 1], tp[:], axis=X, op=MIN
                    )

            # ---- epilogue ----
            # staged into one SBUF tile [conf | p2g | g2p] whose final
            # producer is always the DVE, so the single output DMA has one wait
            out_sb = SP.tile([128, 2 * npt + ngt], f32, tag="out_sb")
            nc.vector.tensor_scalar_max(p2g_min[:], p2g_min[:], 0.0)
            ep = SP.tile([128, npt], f32, tag="ep")
            # sqrt(10000*x) == 100*sqrt(x)
            nc.scalar.activation(ep[:], p2g_min[:], AF.Sqrt, scale=10000.0)
            lnc = SP.tile([128, npt], f32, tag="lnc")
            nc.scalar.activation(lnc[:], conf_ep[:], AF.Ln)
            nc.vector.tensor_tensor(
                out_sb[:, npt : 2 * npt], ep[:], mask_ep[:], op=MUL
            )
            o2 = SP.tile([128, npt], f32, tag="o2")
            nc.vector.tensor_tensor(o2[:], ep[:], conf_ep[:], op=MUL)
            nc.vector.tensor_tensor(o2[:], o2[:], lnc[:], op=SUB)
            nc.vector.tensor_tensor(out_sb[:, 0:npt], o2[:], mask_ep[:], op=MUL)

            nc.vector.tensor_scalar_max(g2p_min[:], g2p_min[:], 0.0)
            g2 = SP.tile([128, ngt], f32, tag="g2")
            nc.scalar.activation(g2[:], g2p_min[:], AF.Sqrt, scale=10000.0)
            nc.vector.tensor_copy(out_sb[:, 2 * npt :], g2[:])
            nc.sync.dma_start(o_all[:, :], out_sb[:])

    nc.compile()
    return nc


def _build16(v1, v2c, mm_dtype_name="float32", repeat=1, split16=False):
    """fp16 reduction-path variant: the K=5 matmul still runs in
    fp32(+/-r) with exact fp32 PSUM, but each PSUM tile is downconverted
    once by the ScalarE to fp16 in SBUF, so both DVE min-folds run in the
    2x_1P perf mode (2 elem/cycle/lane) instead of fp32-PSUM 1x.
    Cost: one fp16 rounding of d2 (~5e-4 relative) before the min."""
    import concourse.tile as tile
    from concourse import bacc, mybir

    f32 = mybir.dt.float32
    f16 = mybir.dt.float16
    mm_dt = getattr(mybir.dt, mm_dtype_name)
    MIN = mybir.AluOpType.min
    MUL = mybir.AluOpType.mult
    SUB = mybir.AluOpType.subtract
    X = mybir.AxisListType.X
    AF = mybir.ActivationFunctionType

    npt = v2c // 128  # pred tiles
    W = min(2048, v1)  # wide tile: up to 4 matmul chunks, one 4-bank PSUM tile
    ng = v1 // W  # wide gt groups
    nblk = W // 32  # 32-wide blocks per group (DVE transpose)
    BIG16 = 60000.0
    ow = 2 * npt + ng * nblk  # fused output width

    nc = bacc.Bacc()
    S = v2c + v1
    if split16:
        # fp16 hi/lo split operands: d2 = A_hi.G_hi + A_hi.G_lo + A_lo.G_hi
        # (3 fp16 matmuls at 1 cyc/row, PSUM-accumulated; dropped lo.lo
        # term is ~2^-24 relative)
        mm_dt = f16
        ag_in = nc.dram_tensor("ag", [5, 2 * S], f16, kind="ExternalInput")
    else:
        ag_in = nc.dram_tensor("ag", [5, S], mm_dt, kind="ExternalInput")
    mc_in = nc.dram_tensor("mc", [128, 2 * npt], f32, kind="ExternalInput")
    o_all = nc.dram_tensor("o_all", [128, ow], f32, kind="ExternalOutput")

    with tile.TileContext(nc) as tc:
        with (
            tc.tile_pool(name="persist", bufs=1) as P,
            tc.tile_pool(name="rowp", bufs=2) as RP,
            tc.tile_pool(name="s16p", bufs=3) as S16P,
            tc.tile_pool(name="small", bufs=1) as SP,
            tc.tile_pool(name="mmps", bufs=2, space="PSUM") as MMPS,
        ):
            AG = P.tile([5, (2 * S if split16 else S)], mm_dt, tag="AG")
            A = AG[:, 0:v2c]
            G = AG[:, v2c:S]
            A_lo = AG[:, S : S + v2c] if split16 else None
            G_lo = AG[:, S + v2c : 2 * S] if split16 else None
            MC = P.tile([128, 2 * npt], f32, tag="MC")
            mc_sb = P.tile([128, 2 * npt], f32, tag="mc_sb")
            mask_ep = mc_sb[:, 0:npt]
            conf_ep = mc_sb[:, npt : 2 * npt]
            colacc = [
                P.tile([128, W], f16, tag=f"col{g}", name=f"col{g}")
                for g in range(ng)
            ]
            p2g_min = P.tile([128, npt], f32, tag="p2gmin")
            g2p_min = P.tile([32, ng * nblk], f32, tag="g2pmin")

            nc.sync.dma_start(AG[:], ag_in[:, :])
            nc.sync.dma_start(MC[:], mc_in[:, :])
            nc.vector.tensor_copy(mc_sb[:], MC[:])

            for g in range(ng):
                nc.vector.memset(colacc[g][:], BIG16)

            # ---- main loop ----
            for pt in [p for _ in range(repeat) for p in range(npt)]:
                rowacc = RP.tile([128, W], f16, tag="rowacc")
                psl = slice(pt * 128, (pt + 1) * 128)
                lhsT = A[:, psl]
                for g in range(ng):
                    ps = MMPS.tile([128, W], f32, tag="mm")
                    for i in range(W // 512):
                        csl = slice((g * 4 + i) * 512, (g * 4 + i + 1) * 512)
                        if split16:
                            nc.tensor.matmul(
                                ps[:, i * 512 : (i + 1) * 512],
                                lhsT, G[:, csl], start=True, stop=False,
                            )
                            nc.tensor.matmul(
                                ps[:, i * 512 : (i + 1) * 512],
                                lhsT, G_lo[:, csl], start=False, stop=False,
                            )
                            nc.tensor.matmul(
                                ps[:, i * 512 : (i + 1) * 512],
                                A_lo[:, psl], G[:, csl], start=False, stop=True,
                            )
                        else:
                            nc.tensor.matmul(
                                ps[:, i * 512 : (i + 1) * 512],
                                lhsT, G[:, csl], start=True, stop=True,
                            )
                    s16 = S16P.tile([128, W], f16, tag="s16")
                    nc.scalar.copy(s16[:], ps[:])
                    if g == 0:
                        nc.vector.tensor_copy(rowacc[:], s16[:])
                    else:
                        nc.vector.tensor_tensor(rowacc[:], rowacc[:], s16[:], op=MIN)
                    nc.vector.tensor_tensor(
                        colacc[g][:], colacc[g][:], s16[:], op=MIN
                    )
                nc.vector.tensor_reduce(
                    p2g_min[:, pt : pt + 1], rowacc[:], axis=X, op=MIN
                )

            # ---- column (gt2pred) finish, DVE + DMA realign ----
            # 32x32 block transpose + free reduce gives r128[32a+i, b] =
            # min over one partition quarter; DMA realigns quarters to base
            # partition 0 so the final folds have equal base partitions
            # (TT with both SBUF inputs requires equal bases).
            K_ = ng * nblk
            r128 = P.tile([128, K_], f16, tag="r128")
            for g in range(ng):
                tr = SP.tile([128, W], f16, tag="tr", name=f"tr{g}")
                nc.vector.transpose(tr[:], colacc[g][:])
                nc.vector.tensor_reduce(
                    r128[:, g * nblk : (g + 1) * nblk],
                    tr[:].rearrange("p (b j) -> p b j", j=32),
                    axis=X,
                    op=MIN,
                )
            r2 = P.tile([32, 3 * K_], f16, tag="r2")
            for a in range(1, 4):
                nc.sync.dma_start(
                    r2[:, (a - 1) * K_ : a * K_], r128[32 * a : 32 * (a + 1), :]
                )
            g2pm16 = P.tile([32, K_], f16, tag="g2pm16")
            nc.vector.tensor_tensor(g2pm16[:], r128[0:32, :], r2[:, 0:K_], op=MIN)
            nc.vector.tensor_tensor(g2pm16[:], g2pm16[:], r2[:, K_ : 2 * K_], op=MIN)
            nc.vector.tensor_tensor(
                g2pm16[:], g2pm16[:], r2[:, 2 * K_ : 3 * K_], op=MIN
            )
            nc.vector.tensor_copy(g2p_min[:], g2pm16[:])

            # ---- epilogue ----
            out_sb = SP.tile([128, ow], f32, tag="out_sb")
            nc.vector.memset(out_sb[:], 0.0)
            nc.vector.tensor_scalar_max(p2g_min[:], p2g_min[:], 0.0)
            ep = SP.tile([128, npt], f32, tag="ep")
            nc.scalar.activation(ep[:], p2g_min[:], AF.Sqrt, scale=10000.0)
            lnc = SP.tile([128, npt], f32, tag="lnc")
            nc.scalar.activation(lnc[:], conf_ep[:], AF.Ln)
            nc.vector.tensor_tensor(
                out_sb[:, npt : 2 * npt], ep[:], mask_ep[:], op=MUL
            )
            o2 = SP.tile([128, npt], f32, tag="o2")
            nc.vector.tensor_tensor(o2[:], ep[:], conf_ep[:], op=MUL)
            nc.vector.tensor_tensor(o2[:], o2[:], lnc[:], op=SUB)
            nc.vector.tensor_tensor(out_sb[:, 0:npt], o2[:], mask_ep[:], op=MUL)

            nc.vector.tensor_scalar_max(g2p_min[:], g2p_min[:], 0.0)
            g2 = SP.tile([32, ng * nblk], f32, tag="g2")
            nc.scalar.activation(g2[:], g2p_min[:], AF.Sqrt, scale=10000.0)
            nc.vector.tensor_copy(out_sb[0:32, 2 * npt :], g2[:])
            nc.sync.dma_start(o_all[:, :], out_sb[:])

    nc.compile()
    return nc


def get_nc(v1=V1, v2c=V2C, mm_dtype_name="float32", repeat=1, variant="f32"):
    key = (v1, v2c, mm_dtype_name, repeat, variant)
    if key not in _BUILT:
        if variant == "f16x2":
            _BUILT[key] = _build16(v1, v2c, mm_dtype_name, repeat, split16=True)
        elif variant == "f16":
            _BUILT[key] = _build16(v1, v2c, mm_dtype_name, repeat)
        else:
            _BUILT[key] = _build(v1, v2c, mm_dtype_name, repeat)
    return _BUILT[key]


def make_aug(gt, xp):
    """Fused augmented matmul operand [A | G]: one K=5 matmul yields the
    full squared-distance expansion |xp|^2 + |gt|^2 - 2 xp.gt."""
    v2c = xp.shape[0]
    v1 = gt.shape[0]
    ag = np.empty((5, v2c + v1), np.float32)
    ag[0:3, :v2c] = -2.0 * xp.T
    ag[3, :v2c] = (xp * xp).sum(-1)
    ag[4, :v2c] = 1.0
    ag[0:3, v2c:] = gt.T
    ag[3, v2c:] = 1.0
    ag[4, v2c:] = (gt * gt).sum(-1)
    return ag


def make_in_maps(x_gt, x_pred, mask, confidence, split16=False):
    """Shard full inputs into per-core input maps (host-side layout only)."""
    npt = V2C // 128
    in_maps = []
    for c in range(N_CORES):
        b, s = divmod(c, SLICES)
        sl = slice(s * V2C, (s + 1) * V2C)
        xp = x_pred[b, sl] * mask[b, sl, None]  # (V2C, 3) masked preds
        m = mask[b, sl]
        cf = confidence[b, sl]
        ag = make_aug(x_gt[b], xp)
        if split16:
            hi = ag.astype(np.float16)
            lo = (ag - hi.astype(np.float32)).astype(np.float16)
            ag = np.concatenate([hi, lo], axis=1)
        mc = np.empty((128, 2 * npt), np.float32)
        mc[:, :npt] = m.reshape(npt, 128).T
        mc[:, npt:] = cf.reshape(npt, 128).T
        in_maps.append({"ag": ag, "mc": mc})
    return in_maps


def assemble_outputs(results):
    """Gather per-core outputs back to full shapes."""
    loss_conf = np.empty((B, V2), dtype=np.float32)
    loss_p2g = np.empty((B, V2), dtype=np.float32)
    loss_g2p = np.full((B, V1), np.inf, dtype=np.float32)
    for c in range(N_CORES):
        b, s = divmod(c, SLICES)
        sl = slice(s * V2C, (s + 1) * V2C)
        npt = V2C // 128
        o = results[c]["o_all"]
        loss_conf[b, sl] = o[:, 0:npt].T.reshape(V2C)
        loss_p2g[b, sl] = o[:, npt : 2 * npt].T.reshape(V2C)
        if o.shape[1] == 2 * npt + V1 // 128:
            part = o[:, 2 * npt :].T.reshape(V1)  # f32 variant: [p, gtile]
        else:
            # f16 variant: [i, g*64+b] -> gt = g*2048 + 32*b + i
            part = o[0:32, 2 * npt :].T.reshape(V1)
        np.minimum(loss_g2p[b], part, out=loss_g2p[b])
    return loss_conf, loss_p2g, loss_g2p


def kernel(x_gt, x_pred, mask, confidence):
    from concourse.bass_utils import run_bass_kernel_spmd

    nc = get_nc()
    in_maps = make_in_maps(
        np.asarray(x_gt), np.asarray(x_pred), np.asarray(mask), np.asarray(confidence)
    )
    res = run_bass_kernel_spmd(nc, in_maps, list(range(N_CORES)))
    return assemble_outputs(res.results)



# revision 6
# speedup vs baseline: 3.5265x; 3.5265x over previous
"""Bidirectional chamfer loss kernel for Trainium2 (8 NeuronCores).

Problem (hardcoded): B=2 batches, V1=8192 gt points, V2=8192 pred points, 3D.
  d2[b,i,j] = max(0, |xp_i|^2 + |gt_j|^2 - 2 xp_i.gt_j),  xp = x_pred * mask
  loss_pred2gt[b,i] = sqrt(min_j d2) * 100
  loss_gt2pred[b,j] = sqrt(min_i d2) * 100
  loss_conf = (loss_pred2gt * conf - ln(conf)) * mask ; loss_pred2gt *= mask

Sharding: 8 cores = 2 batches x 4 V2-slices (2048 preds/core vs full 8192 gt).
Each core computes its pred2gt slice exactly, and a partial gt2pred
(min over its preds); the host combines partials with np.minimum.

Mask compaction (host): masked preds all collapse to the origin and their
pred2gt outputs are zeroed by the reference, so each core only processes
its ACTIVE preds, compacted to the front and padded to V2CE=1792 slots
(14 tiles of 128; ~1638 active expected from an 80% mask). Pad slots are
origin points — exactly what masked preds become — so the gt2pred fold
stays correct; their pred2gt outputs are discarded on the host. If a
slice somehow has more active preds than V2CE, a larger variant is
compiled on the fly.

Device kernel V2 (per core, SPMD):
  - PE: ONE K=13 fp16 matmul per (pred-tile 128, gt-chunk 512). The fp16
    hi/lo split of the distance expansion is K-packed into a single
    matmul (PE cost depends only on the moving free dim, not K):
      rows 0-2:  a_hi(xyz)   x g_hi(xyz)
      rows 3-5:  a_hi(xyz)   x g_lo(xyz)
      rows 6-8:  a_lo(xyz)   x g_hi(xyz)
      rows 9-10: |xp|^2 hi/lo x 1
      rows 11-12: 1           x |gt|^2 hi/lo
    (a = -2*xp). fp16 products are exact in fp32 PSUM; the dropped
    lo*lo term is ~2^-22 relative -> d2 is near-exact.
  - Scalar: converts each PSUM tile [128,2048] to NEGATED fp16 in SBUF
    (activation Copy, scale=-1). Negation turns all min-reductions into
    max-reductions so the gt2pred partition reduce can use gpsimd
    partition_all_reduce (which supports max but not min).
  - DVE: per pred-tile row max-tree over the negated fp16 data (fp16
    tensor_tensor runs in 2x_1p mode), plus a column max-fold over the
    first JD gt columns.
  - GpSimd (Pool): column max-fold over the remaining gt columns during
    the main loop, then partition_all_reduce(max) over both column
    accumulators as the tail.
  - Epilogue: clamp, sqrt(-10000*x) == 100*sqrt(d2), conf math.

Outputs per core: o_all [128, 2*npt] = [conf_loss | p2g_loss] in the
(npt,128)->tile-major layout, and o_g2p [1, v1] = gt2pred partials.
"""

import numpy as np

B = 2
V1 = 8192  # gt points
V2 = 8192  # pred points (total)
N_CORES = 8
SLICES = N_CORES // B  # V2-slices per batch
V2C = V2 // SLICES  # pred points per core (before compaction)
V2CE = 1792  # compacted+padded pred slots per core

KROWS = 13

_BUILT = {}


def _jd(v1):
    """DVE share of the gt columns for the column fold (rest on gpsimd)."""
    return max(512, (v1 * 9 // 16) // 512 * 512)


def _build_v2(v1, v2c, repeat=1, jd=None):
    import concourse.tile as tile
    from concourse import bacc, bass_isa, mybir

    f32 = mybir.dt.float32
    f16 = mybir.dt.float16
    MAX = mybir.AluOpType.max
    MUL = mybir.AluOpType.mult
    SUB = mybir.AluOpType.subtract
    X = mybir.AxisListType.X
    AF = mybir.ActivationFunctionType

    npt = v2c // 128  # pred tiles
    W = min(2048, v1)  # PSUM super-tile width (4 banks)
    ng = v1 // W  # super-tiles per pred tile
    S = v2c + v1

    nc = bacc.Bacc()
    ag_in = nc.dram_tensor("ag", [KROWS, S], f16, kind="ExternalInput")
    mc_in = nc.dram_tensor("mc", [128, npt], f32, kind="ExternalInput")
    o_all = nc.dram_tensor("o_all", [128, 2 * npt], f32, kind="ExternalOutput")
    o_g2p = nc.dram_tensor("o_g2p", [1, v1], f32, kind="ExternalOutput")

    with tile.TileContext(nc) as tc:
        with (
            tc.tile_pool(name="persist", bufs=1) as P,
            tc.tile_pool(name="s16p", bufs=2) as S16P,
            tc.tile_pool(name="rowp", bufs=2) as RP,
            tc.tile_pool(name="small", bufs=1) as SP,
            tc.tile_pool(name="mmps", bufs=2, space="PSUM") as MMPS,
        ):
            AG = P.tile([KROWS, S], f16, tag="AG")
            A = AG[:, 0:v2c]
            G = AG[:, v2c:S]
            MC = P.tile([128, npt], f32, tag="MC")
            conf_ep = P.tile([128, npt], f32, tag="mc_sb")
            colD = P.tile([128, v1], f16, tag="colD")
            p2g_neg = P.tile([128, npt], f32, tag="p2gneg")

            nc.sync.dma_start(AG[:], ag_in[:, :])
            nc.sync.dma_start(MC[:], mc_in[:, :])
            nc.vector.tensor_copy(conf_ep[:], MC[:])

            # ---- main loop ----
            for rep in range(repeat):
                for pt in range(npt):
                    s16 = S16P.tile([128, v1], f16, tag="s16")
                    psl = slice(pt * 128, (pt + 1) * 128)
                    for g in range(ng):
                        ps = MMPS.tile([128, W], f32, tag="mm")
                        for i in range(W // 512):
                            c0 = g * W + i * 512
                            nc.tensor.matmul(
                                ps[:, i * 512 : (i + 1) * 512],
                                A[:, psl],
                                G[:, c0 : c0 + 512],
                                start=True,
                                stop=True,
                            )
                        # PSUM f32 -> negated fp16 SBUF
                        nc.scalar.activation(
                            s16[:, g * W : (g + 1) * W], ps[:], AF.Copy, scale=-1.0
                        )
                    # row max-tree (negated data): fp16 TT runs 2x_1p
                    w = v1
                    src = s16
                    lvl = 0
                    while w > 512:
                        h = w // 2
                        dst = RP.tile([128, h], f16, tag=f"t{lvl}")
                        nc.vector.tensor_tensor(
                            dst[:], src[:, 0:h], src[:, h:w], op=MAX
                        )
                        src, w, lvl = dst, h, lvl + 1
                    nc.vector.tensor_reduce(
                        p2g_neg[:, pt : pt + 1], src[:, 0:w], axis=X, op=MAX
                    )
                    # column fold (first pt of a pass initializes by copy)
                    if pt == 0:
                        nc.vector.tensor_copy(colD[:], s16[:])
                    else:
                        nc.vector.tensor_tensor(
                            colD[:], colD[:], s16[:], op=MAX
                        )

            # ---- gt2pred tail ----
            g2p16 = SP.tile([128, v1], f16, tag="g2p16")
            nc.gpsimd.partition_all_reduce(
                g2p16[:], colD[:], 128, bass_isa.ReduceOp.max
            )
            g2prow = g2p16[0:1, :]
            nc.vector.tensor_scalar_min(g2prow, g2prow, 0.0)
            g2f = SP.tile([1, v1], f32, tag="g2f")
            # sqrt(-10000 * (-d2)) == 100*sqrt(d2)
            nc.scalar.activation(g2f[:], g2prow, AF.Sqrt, scale=-10000.0)
            nc.sync.dma_start(o_g2p[:, :], g2f[:])

            # ---- pred2gt epilogue (compacted preds are all active) ----
            out_sb = SP.tile([128, 2 * npt], f32, tag="out_sb")
            nc.vector.tensor_scalar_min(p2g_neg[:], p2g_neg[:], 0.0)
            ep = out_sb[:, npt : 2 * npt]
            nc.scalar.activation(ep, p2g_neg[:], AF.Sqrt, scale=-10000.0)
            lnc = SP.tile([128, npt], f32, tag="lnc")
            nc.scalar.activation(lnc[:], conf_ep[:], AF.Ln)
            o2 = SP.tile([128, npt], f32, tag="o2")
            nc.vector.tensor_tensor(o2[:], ep, conf_ep[:], op=MUL)
            nc.vector.tensor_tensor(out_sb[:, 0:npt], o2[:], lnc[:], op=SUB)
            nc.sync.dma_start(o_all[:, :], out_sb[:])

    nc.compile()
    return nc


def get_nc(v1=V1, v2c=V2CE, repeat=1, jd=None):
    key = (v1, v2c, repeat, jd)
    if key not in _BUILT:
        _BUILT[key] = _build_v2(v1, v2c, repeat, jd=jd)
    return _BUILT[key]


def _split16(x):
    hi = x.astype(np.float16)
    lo = (x - hi.astype(np.float32)).astype(np.float16)
    return hi, lo


def make_aug(gt, xp):
    """K=13 fp16 hi/lo-split matmul operand [A | G]; one matmul yields the
    near-exact squared-distance expansion |xp|^2 + |gt|^2 - 2 xp.gt."""
    v2c = xp.shape[0]
    v1 = gt.shape[0]
    a = (-2.0 * xp.T).astype(np.float32)  # (3, v2c)
    ah, al = _split16(a)
    nph, npl = _split16((xp * xp).sum(-1).astype(np.float32))
    g = gt.T.astype(np.float32)  # (3, v1)
    gh, gl = _split16(g)
    ngh, ngl = _split16((gt * gt).sum(-1).astype(np.float32))

    ag = np.zeros((KROWS, v2c + v1), np.float16)
    A = ag[:, :v2c]
    G = ag[:, v2c:]
    A[0:3] = ah
    A[3:6] = ah
    A[6:9] = al
    A[9] = nph
    A[10] = npl
    A[11] = 1.0
    A[12] = 1.0
    G[0:3] = gh
    G[3:6] = gl
    G[6:9] = gh
    G[9] = 1.0
    G[10] = 1.0
    G[11] = ngh
    G[12] = ngl
    return ag


def _compact_core(x_gt_b, x_pred_s, m_s, cf_s, v2c_eff):
    """Compact one core's active preds to the front, pad with origin points
    (the masked-pred equivalent) or a real pred if nothing is masked."""
    idx = np.flatnonzero(m_s > 0.0)
    n = len(idx)
    if n > v2c_eff:
        return None  # caller retries with a bigger variant
    xpc = np.zeros((v2c_eff, 3), np.float32)
    xpc[:n] = x_pred_s[idx]
    cfc = np.ones(v2c_eff, np.float32)
    cfc[:n] = cf_s[idx]
    if n == len(m_s) and n < v2c_eff:
        # no masked preds in this slice: origin may not be a valid pred,
        # pad with a duplicate of the first real pred instead
        xpc[n:] = xpc[0]
    return idx, xpc, cfc


def make_in_maps(x_gt, x_pred, mask, confidence, v2c_eff=V2CE):
    """Shard + compact full inputs into per-core input maps."""
    npt = v2c_eff // 128
    in_maps = []
    for c in range(N_CORES):
        b, s = divmod(c, SLICES)
        sl = slice(s * V2C, (s + 1) * V2C)
        comp = _compact_core(
            x_gt[b], x_pred[b, sl], mask[b, sl], confidence[b, sl], v2c_eff
        )
        assert comp is not None, "active pred overflow; use bigger v2c_eff"
        idx, xpc, cfc = comp
        ag = make_aug(x_gt[b], xpc)
        mc = cfc.reshape(npt, 128).T.copy()
        in_maps.append({"ag": ag, "mc": mc})
    return in_maps


def assemble_outputs(results, mask, v2c_eff=V2CE):
    """Gather per-core outputs back to full shapes (scatter compacted)."""
    npt = v2c_eff // 128
    loss_conf = np.zeros((B, V2), dtype=np.float32)
    loss_p2g = np.zeros((B, V2), dtype=np.float32)
    loss_g2p = np.full((B, V1), np.inf, dtype=np.float32)
    for c in range(N_CORES):
        b, s = divmod(c, SLICES)
        o = results[c]["o_all"]
        idx = np.flatnonzero(mask[b, s * V2C : (s + 1) * V2C] > 0.0)
        n = len(idx)
        pos = s * V2C + idx
        loss_conf[b, pos] = o[:, 0:npt].T.reshape(v2c_eff)[:n]
        loss_p2g[b, pos] = o[:, npt : 2 * npt].T.reshape(v2c_eff)[:n]
        part = results[c]["o_g2p"].reshape(V1)
        np.minimum(loss_g2p[b], part, out=loss_g2p[b])
    return loss_conf, loss_p2g, loss_g2p


def kernel(x_gt, x_pred, mask, confidence):
    from concourse.bass_utils import run_bass_kernel_spmd

    x_gt = np.asarray(x_gt)
    x_pred = np.asarray(x_pred)
    mask = np.asarray(mask)
    confidence = np.asarray(confidence)

    v2c_eff = V2CE
    maxn = max(
        int((mask[b, s * V2C : (s + 1) * V2C] > 0).sum())
        for b in range(B)
        for s in range(SLICES)
    )
    if maxn > v2c_eff:
        v2c_eff = -(-maxn // 128) * 128

    nc = get_nc(v2c=v2c_eff)
    in_maps = make_in_maps(x_gt, x_pred, mask, confidence, v2c_eff)
    res = run_bass_kernel_spmd(nc, in_maps, list(range(N_CORES)))
    return assemble_outputs(res.results, mask, v2c_eff)


# revision 10
# speedup vs baseline: 3.8999x; 1.1059x over previous
"""Bidirectional chamfer loss kernel for Trainium2 (8 NeuronCores).

Problem (hardcoded): B=2 batches, V1=8192 gt points, V2=8192 pred points, 3D.
  d2[b,i,j] = max(0, |xp_i|^2 + |gt_j|^2 - 2 xp_i.gt_j),  xp = x_pred * mask
  loss_pred2gt[b,i] = sqrt(min_j d2) * 100
  loss_gt2pred[b,j] = sqrt(min_i d2) * 100
  loss_conf = (loss_pred2gt * conf - ln(conf)) * mask ; loss_pred2gt *= mask

Sharding: 8 cores = 2 batches x 4 V2-slices (2048 preds/core vs full 8192 gt).
Each core computes its pred2gt slice exactly, and a partial gt2pred
(min over its preds); the host combines partials with np.minimum.

Mask compaction (host): masked preds all collapse to the origin and their
pred2gt outputs are zeroed by the reference, so each core only processes
its ACTIVE preds, compacted to the front and padded to V2CE=1792 slots
(14 tiles of 128; ~1638 active expected from an 80% mask). Pad slots are
origin points — exactly what masked preds become — so the gt2pred fold
stays correct; their pred2gt outputs are discarded on the host. If a
slice somehow has more active preds than V2CE, a larger variant is
compiled on the fly.

Device kernel V2 (per core, SPMD):
  - PE: ONE K=13 fp16 matmul per (pred-tile 128, gt-chunk 512). The fp16
    hi/lo split of the distance expansion is K-packed into a single
    matmul (PE cost depends only on the moving free dim, not K):
      rows 0-2:  a_hi(xyz)   x g_hi(xyz)
      rows 3-5:  a_hi(xyz)   x g_lo(xyz)
      rows 6-8:  a_lo(xyz)   x g_hi(xyz)
      rows 9-10: |xp|^2 hi/lo x 1
      rows 11-12: 1           x |gt|^2 hi/lo
    (a = -2*xp). fp16 products are exact in fp32 PSUM; the dropped
    lo*lo term is ~2^-22 relative -> d2 is near-exact.
  - Scalar: converts each PSUM tile [128,2048] to NEGATED fp16 in SBUF
    (activation Copy, scale=-1). Negation turns all min-reductions into
    max-reductions so the gt2pred partition reduce can use gpsimd
    partition_all_reduce (which supports max but not min).
  - DVE: per pred-tile row max-tree over the negated fp16 data (fp16
    tensor_tensor runs in 2x_1p mode), plus a column max-fold over the
    first JD gt columns.
  - GpSimd (Pool): column max-fold over the remaining gt columns during
    the main loop, then partition_all_reduce(max) over both column
    accumulators as the tail.
  - Epilogue: clamp, sqrt(-10000*x) == 100*sqrt(d2), conf math.

Outputs per core: o_all [128, 2*npt] = [conf_loss | p2g_loss] in the
(npt,128)->tile-major layout, and o_g2p [1, v1] = gt2pred partials.
"""

import numpy as np

B = 2
V1 = 8192  # gt points
V2 = 8192  # pred points (total)
N_CORES = 8
SLICES = N_CORES // B  # V2-slices per batch
V2C = V2 // SLICES  # pred points per core (before compaction)
V2CE = 1792  # compacted+padded pred slots per core

KROWS = 13

_BUILT = {}


def _jd(v1):
    """DVE share of the gt columns for the column fold (rest on gpsimd)."""
    return max(512, (v1 * 9 // 16) // 512 * 512)


def _build_v2(v1, v2c, repeat=1, jd=None):
    import concourse.tile as tile
    from concourse import bacc, bass_isa, mybir

    f32 = mybir.dt.float32
    f16 = mybir.dt.float16
    MAX = mybir.AluOpType.max
    MUL = mybir.AluOpType.mult
    SUB = mybir.AluOpType.subtract
    X = mybir.AxisListType.X
    AF = mybir.ActivationFunctionType

    npt = v2c // 128  # pred tiles
    W = min(2048, v1)  # PSUM super-tile width (4 banks)
    ng = v1 // W  # super-tiles per pred tile
    S = v2c + v1

    nc = bacc.Bacc()
    ag_in = nc.dram_tensor("ag", [KROWS, S], f16, kind="ExternalInput")
    mc_in = nc.dram_tensor("mc", [128, npt], f32, kind="ExternalInput")
    o_all = nc.dram_tensor("o_all", [128, 2 * npt], f32, kind="ExternalOutput")
    o_g2p = nc.dram_tensor("o_g2p", [1, v1], f32, kind="ExternalOutput")

    with tile.TileContext(nc) as tc:
        with (
            tc.tile_pool(name="persist", bufs=1) as P,
            tc.tile_pool(name="s16p", bufs=3) as S16P,
            tc.tile_pool(name="rowp", bufs=2) as RP,
            tc.tile_pool(name="small", bufs=1) as SP,
            tc.tile_pool(name="mmps", bufs=2, space="PSUM") as MMPS,
        ):
            AG = P.tile([KROWS, S], f16, tag="AG")
            A = AG[:, 0:v2c]
            G = AG[:, v2c:S]
            MC = P.tile([128, npt], f32, tag="MC")
            conf_ep = P.tile([128, npt], f32, tag="mc_sb")
            colD = P.tile([128, v1], f16, tag="colD")
            rowbuf = P.tile([128, npt * 512], f16, tag="rowbuf")
            p2g_neg = P.tile([128, npt], f32, tag="p2gneg")

            nc.sync.dma_start(AG[:], ag_in[:, :])
            nc.sync.dma_start(MC[:], mc_in[:, :])
            nc.vector.tensor_copy(conf_ep[:], MC[:])

            # ---- main loop ----
            for rep in range(repeat):
                for pt in range(npt):
                    s16 = S16P.tile([128, v1], f16, tag="s16")
                    psl = slice(pt * 128, (pt + 1) * 128)
                    for g in range(ng):
                        ps = MMPS.tile([128, W], f32, tag="mm")
                        for i in range(W // 512):
                            c0 = g * W + i * 512
                            nc.tensor.matmul(
                                ps[:, i * 512 : (i + 1) * 512],
                                A[:, psl],
                                G[:, c0 : c0 + 512],
                                start=True,
                                stop=True,
                            )
                        # PSUM f32 -> negated fp16 SBUF
                        nc.scalar.activation(
                            s16[:, g * W : (g + 1) * W], ps[:], AF.Copy, scale=-1.0
                        )
                    # row max-tree (negated data): fp16 TT runs 2x_1p;
                    # last level lands in rowbuf for one batched reduce
                    w = v1
                    src = s16
                    lvl = 0
                    while w > 1024:
                        h = w // 2
                        dst = RP.tile([128, h], f16, tag=f"t{lvl}")
                        nc.vector.tensor_tensor(
                            dst[:], src[:, 0:h], src[:, h:w], op=MAX
                        )
                        src, w, lvl = dst, h, lvl + 1
                    h = w // 2
                    nc.vector.tensor_tensor(
                        rowbuf[:, pt * 512 : pt * 512 + h],
                        src[:, 0:h],
                        src[:, h:w],
                        op=MAX,
                    )
                    # column fold (first pt of a pass initializes by copy)
                    if pt == 0:
                        nc.vector.tensor_copy(colD[:], s16[:])
                    else:
                        nc.vector.tensor_tensor(
                            colD[:], colD[:], s16[:], op=MAX
                        )
                # one batched row reduce per pass
                nc.vector.tensor_reduce(
                    p2g_neg[:],
                    rowbuf[:].rearrange("p (t j) -> p t j", j=512),
                    axis=X,
                    op=MAX,
                )

            # ---- gt2pred tail ----
            g2p16 = SP.tile([128, v1], f16, tag="g2p16")
            nc.gpsimd.partition_all_reduce(
                g2p16[:], colD[:], 128, bass_isa.ReduceOp.max
            )
            g2prow = g2p16[0:1, :]
            nc.vector.tensor_scalar_min(g2prow, g2prow, 0.0)
            g2f = SP.tile([1, v1], f32, tag="g2f")
            # sqrt(-10000 * (-d2)) == 100*sqrt(d2)
            nc.scalar.activation(g2f[:], g2prow, AF.Sqrt, scale=-10000.0)
            nc.sync.dma_start(o_g2p[:, :], g2f[:])

            # ---- pred2gt epilogue (compacted preds are all active) ----
            out_sb = SP.tile([128, 2 * npt], f32, tag="out_sb")
            nc.vector.tensor_scalar_min(p2g_neg[:], p2g_neg[:], 0.0)
            ep = out_sb[:, npt : 2 * npt]
            nc.scalar.activation(ep, p2g_neg[:], AF.Sqrt, scale=-10000.0)
            lnc = SP.tile([128, npt], f32, tag="lnc")
            nc.scalar.activation(lnc[:], conf_ep[:], AF.Ln)
            o2 = SP.tile([128, npt], f32, tag="o2")
            nc.vector.tensor_tensor(o2[:], ep, conf_ep[:], op=MUL)
            nc.vector.tensor_tensor(out_sb[:, 0:npt], o2[:], lnc[:], op=SUB)
            nc.sync.dma_start(o_all[:, :], out_sb[:])

    nc.compile()
    return nc


def get_nc(v1=V1, v2c=V2CE, repeat=1, jd=None):
    key = (v1, v2c, repeat, jd)
    if key not in _BUILT:
        _BUILT[key] = _build_v2(v1, v2c, repeat, jd=jd)
    return _BUILT[key]


def _split16(x):
    hi = x.astype(np.float16)
    lo = (x - hi.astype(np.float32)).astype(np.float16)
    return hi, lo


def make_aug(gt, xp):
    """K=13 fp16 hi/lo-split matmul operand [A | G]; one matmul yields the
    near-exact squared-distance expansion |xp|^2 + |gt|^2 - 2 xp.gt."""
    v2c = xp.shape[0]
    v1 = gt.shape[0]
    a = (-2.0 * xp.T).astype(np.float32)  # (3, v2c)
    ah, al = _split16(a)
    nph, npl = _split16((xp * xp).sum(-1).astype(np.float32))
    g = gt.T.astype(np.float32)  # (3, v1)
    gh, gl = _split16(g)
    ngh, ngl = _split16((gt * gt).sum(-1).astype(np.float32))

    ag = np.zeros((KROWS, v2c + v1), np.float16)
    A = ag[:, :v2c]
    G = ag[:, v2c:]
    A[0:3] = ah
    A[3:6] = ah
    A[6:9] = al
    A[9] = nph
    A[10] = npl
    A[11] = 1.0
    A[12] = 1.0
    G[0:3] = gh
    G[3:6] = gl
    G[6:9] = gh
    G[9] = 1.0
    G[10] = 1.0
    G[11] = ngh
    G[12] = ngl
    return ag


def _compact_core(x_gt_b, x_pred_s, m_s, cf_s, v2c_eff):
    """Compact one core's active preds to the front, pad with origin points
    (the masked-pred equivalent) or a real pred if nothing is masked."""
    idx = np.flatnonzero(m_s > 0.0)
    n = len(idx)
    if n > v2c_eff:
        return None  # caller retries with a bigger variant
    xpc = np.zeros((v2c_eff, 3), np.float32)
    xpc[:n] = x_pred_s[idx]
    cfc = np.ones(v2c_eff, np.float32)
    cfc[:n] = cf_s[idx]
    if n == len(m_s) and n < v2c_eff:
        # no masked preds in this slice: origin may not be a valid pred,
        # pad with a duplicate of the first real pred instead
        xpc[n:] = xpc[0]
    return idx, xpc, cfc


def make_in_maps(x_gt, x_pred, mask, confidence, v2c_eff=V2CE):
    """Shard + compact full inputs into per-core input maps."""
    npt = v2c_eff // 128
    in_maps = []
    for c in range(N_CORES):
        b, s = divmod(c, SLICES)
        sl = slice(s * V2C, (s + 1) * V2C)
        comp = _compact_core(
            x_gt[b], x_pred[b, sl], mask[b, sl], confidence[b, sl], v2c_eff
        )
        assert comp is not None, "active pred overflow; use bigger v2c_eff"
        idx, xpc, cfc = comp
        ag = make_aug(x_gt[b], xpc)
        mc = cfc.reshape(npt, 128).T.copy()
        in_maps.append({"ag": ag, "mc": mc})
    return in_maps


def assemble_outputs(results, mask, v2c_eff=V2CE):
    """Gather per-core outputs back to full shapes (scatter compacted)."""
    npt = v2c_eff // 128
    loss_conf = np.zeros((B, V2), dtype=np.float32)
    loss_p2g = np.zeros((B, V2), dtype=np.float32)
    loss_g2p = np.full((B, V1), np.inf, dtype=np.float32)
    for c in range(N_CORES):
        b, s = divmod(c, SLICES)
        o = results[c]["o_all"]
        idx = np.flatnonzero(mask[b, s * V2C : (s + 1) * V2C] > 0.0)
        n = len(idx)
        pos = s * V2C + idx
        loss_conf[b, pos] = o[:, 0:npt].T.reshape(v2c_eff)[:n]
        loss_p2g[b, pos] = o[:, npt : 2 * npt].T.reshape(v2c_eff)[:n]
        part = results[c]["o_g2p"].reshape(V1)
        np.minimum(loss_g2p[b], part, out=loss_g2p[b])
    return loss_conf, loss_p2g, loss_g2p


def kernel(x_gt, x_pred, mask, confidence):
    from concourse.bass_utils import run_bass_kernel_spmd

    x_gt = np.asarray(x_gt)
    x_pred = np.asarray(x_pred)
    mask = np.asarray(mask)
    confidence = np.asarray(confidence)

    v2c_eff = V2CE
    maxn = max(
        int((mask[b, s * V2C : (s + 1) * V2C] > 0).sum())
        for b in range(B)
        for s in range(SLICES)
    )
    if maxn > v2c_eff:
        v2c_eff = -(-maxn // 128) * 128

    nc = get_nc(v2c=v2c_eff)
    in_maps = make_in_maps(x_gt, x_pred, mask, confidence, v2c_eff)
    res = run_bass_kernel_spmd(nc, in_maps, list(range(N_CORES)))
    return assemble_outputs(res.results, mask, v2c_eff)


# revision 11
# speedup vs baseline: 4.0757x; 1.0451x over previous
"""Bidirectional chamfer loss kernel for Trainium2 (8 NeuronCores).

Problem (hardcoded): B=2 batches, V1=8192 gt points, V2=8192 pred points, 3D.
  d2[b,i,j] = max(0, |xp_i|^2 + |gt_j|^2 - 2 xp_i.gt_j),  xp = x_pred * mask
  loss_pred2gt[b,i] = sqrt(min_j d2) * 100
  loss_gt2pred[b,j] = sqrt(min_i d2) * 100
  loss_conf = (loss_pred2gt * conf - ln(conf)) * mask ; loss_pred2gt *= mask

Sharding: 8 cores = 2 batches x 4 V2-slices (2048 preds/core vs full 8192 gt).
Each core computes its pred2gt slice exactly, and a partial gt2pred
(min over its preds); the host combines partials with np.minimum.

Mask compaction (host): masked preds all collapse to the origin and their
pred2gt outputs are zeroed by the reference, so each core only processes
its ACTIVE preds, compacted to the front and padded to V2CE=1792 slots
(14 tiles of 128; ~1638 active expected from an 80% mask). Pad slots are
origin points — exactly what masked preds become — so the gt2pred fold
stays correct; their pred2gt outputs are discarded on the host. If a
slice somehow has more active preds than V2CE, a larger variant is
compiled on the fly.

Device kernel V2 (per core, SPMD):
  - PE: ONE K=13 fp16 matmul per (pred-tile 128, gt-chunk 512). The fp16
    hi/lo split of the distance expansion is K-packed into a single
    matmul (PE cost depends only on the moving free dim, not K):
      rows 0-2:  a_hi(xyz)   x g_hi(xyz)
      rows 3-5:  a_hi(xyz)   x g_lo(xyz)
      rows 6-8:  a_lo(xyz)   x g_hi(xyz)
      rows 9-10: |xp|^2 hi/lo x 1
      rows 11-12: 1           x |gt|^2 hi/lo
    (a = -2*xp). fp16 products are exact in fp32 PSUM; the dropped
    lo*lo term is ~2^-22 relative -> d2 is near-exact.
  - Scalar: converts each PSUM tile [128,2048] to NEGATED fp16 in SBUF
    (activation Copy, scale=-1). Negation turns all min-reductions into
    max-reductions so the gt2pred partition reduce can use gpsimd
    partition_all_reduce (which supports max but not min).
  - DVE: per pred-tile row max-tree over the negated fp16 data (fp16
    tensor_tensor runs in 2x_1p mode), plus a column max-fold over the
    first JD gt columns.
  - GpSimd (Pool): column max-fold over the remaining gt columns during
    the main loop, then partition_all_reduce(max) over both column
    accumulators as the tail.
  - Epilogue: clamp, sqrt(-10000*x) == 100*sqrt(d2), conf math.

Outputs per core: o_all [128, 2*npt] = [conf_loss | p2g_loss] in the
(npt,128)->tile-major layout, and o_g2p [1, v1] = gt2pred partials.
"""

import numpy as np

B = 2
V1 = 8192  # gt points
V2 = 8192  # pred points (total)
N_CORES = 8
SLICES = N_CORES // B  # V2-slices per batch
V2C = V2 // SLICES  # pred points per core (before compaction)
V2CE = 1792  # compacted+padded pred slots per core

KROWS = 13

_BUILT = {}


def _jd(v1):
    """DVE share of the gt columns for the column fold (rest on gpsimd)."""
    return max(512, (v1 * 9 // 16) // 512 * 512)


def _build_v2(v1, v2c, repeat=1, jd=None):
    import concourse.tile as tile
    from concourse import bacc, bass_isa, mybir

    f32 = mybir.dt.float32
    f16 = mybir.dt.float16
    MAX = mybir.AluOpType.max
    MUL = mybir.AluOpType.mult
    SUB = mybir.AluOpType.subtract
    X = mybir.AxisListType.X
    AF = mybir.ActivationFunctionType

    npt = v2c // 128  # pred tiles
    W = min(2048, v1)  # PSUM super-tile width (4 banks)
    ng = v1 // W  # super-tiles per pred tile
    S = v2c + v1

    nc = bacc.Bacc()
    ag_in = nc.dram_tensor("ag", [KROWS, S], f16, kind="ExternalInput")
    mc_in = nc.dram_tensor("mc", [128, npt], f32, kind="ExternalInput")
    o_all = nc.dram_tensor("o_all", [128, 2 * npt], f32, kind="ExternalOutput")
    o_g2p = nc.dram_tensor("o_g2p", [1, v1], f32, kind="ExternalOutput")

    with tile.TileContext(nc) as tc:
        with (
            tc.tile_pool(name="persist", bufs=1) as P,
            tc.tile_pool(name="s16p", bufs=3) as S16P,
            tc.tile_pool(name="rowp", bufs=2) as RP,
            tc.tile_pool(name="small", bufs=1) as SP,
            tc.tile_pool(name="mmps", bufs=2, space="PSUM") as MMPS,
        ):
            AG = P.tile([KROWS, S], f16, tag="AG")
            A = AG[:, 0:v2c]
            G = AG[:, v2c:S]
            MC = P.tile([128, npt], f32, tag="MC")
            conf_ep = P.tile([128, npt], f32, tag="mc_sb")
            colD = P.tile([128, v1], f16, tag="colD")
            rowbuf = P.tile([128, npt * 512], f16, tag="rowbuf")
            p2g_neg = P.tile([128, npt], f32, tag="p2gneg")

            nc.sync.dma_start(AG[:], ag_in[:, :])
            nc.sync.dma_start(MC[:], mc_in[:, :])
            nc.vector.tensor_copy(conf_ep[:], MC[:])

            # ---- main loop ----
            for rep in range(repeat):
                for pt in range(npt):
                    s16 = S16P.tile([128, v1], f16, tag="s16")
                    psl = slice(pt * 128, (pt + 1) * 128)
                    for g in range(ng):
                        ps = MMPS.tile([128, W], f32, tag="mm")
                        for i in range(W // 512):
                            c0 = g * W + i * 512
                            nc.tensor.matmul(
                                ps[:, i * 512 : (i + 1) * 512],
                                A[:, psl],
                                G[:, c0 : c0 + 512],
                                start=True,
                                stop=True,
                            )
                        # PSUM f32 -> negated fp16 SBUF
                        nc.scalar.activation(
                            s16[:, g * W : (g + 1) * W], ps[:], AF.Copy, scale=-1.0
                        )
                    # row max-tree (negated data): fp16 TT runs 2x_1p;
                    # last level lands in rowbuf for one batched reduce
                    w = v1
                    src = s16
                    lvl = 0
                    while w > 1024:
                        h = w // 2
                        dst = RP.tile([128, h], f16, tag=f"t{lvl}")
                        nc.vector.tensor_tensor(
                            dst[:], src[:, 0:h], src[:, h:w], op=MAX
                        )
                        src, w, lvl = dst, h, lvl + 1
                    h = w // 2
                    nc.vector.tensor_tensor(
                        rowbuf[:, pt * 512 : pt * 512 + h],
                        src[:, 0:h],
                        src[:, h:w],
                        op=MAX,
                    )
                    # column fold (first pt of a pass initializes via a
                    # Scalar-engine copy -- Act has headroom, DVE is the wall)
                    if pt == 0:
                        nc.scalar.copy(colD[:], s16[:])
                    else:
                        nc.vector.tensor_tensor(
                            colD[:], colD[:], s16[:], op=MAX
                        )
                # one batched row reduce per pass
                nc.vector.tensor_reduce(
                    p2g_neg[:],
                    rowbuf[:].rearrange("p (t j) -> p t j", j=512),
                    axis=X,
                    op=MAX,
                )

            # ---- gt2pred tail ----
            g2p16 = SP.tile([128, v1], f16, tag="g2p16")
            nc.gpsimd.partition_all_reduce(
                g2p16[:], colD[:], 128, bass_isa.ReduceOp.max
            )
            g2prow = g2p16[0:1, :]
            nc.vector.tensor_scalar_min(g2prow, g2prow, 0.0)
            g2f = SP.tile([1, v1], f32, tag="g2f")
            # sqrt(-10000 * (-d2)) == 100*sqrt(d2)
            nc.scalar.activation(g2f[:], g2prow, AF.Sqrt, scale=-10000.0)
            nc.sync.dma_start(o_g2p[:, :], g2f[:])

            # ---- pred2gt epilogue (compacted preds are all active) ----
            out_sb = SP.tile([128, 2 * npt], f32, tag="out_sb")
            nc.vector.tensor_scalar_min(p2g_neg[:], p2g_neg[:], 0.0)
            ep = out_sb[:, npt : 2 * npt]
            nc.scalar.activation(ep, p2g_neg[:], AF.Sqrt, scale=-10000.0)
            lnc = SP.tile([128, npt], f32, tag="lnc")
            nc.scalar.activation(lnc[:], conf_ep[:], AF.Ln)
            o2 = SP.tile([128, npt], f32, tag="o2")
            nc.vector.tensor_tensor(o2[:], ep, conf_ep[:], op=MUL)
            nc.vector.tensor_tensor(out_sb[:, 0:npt], o2[:], lnc[:], op=SUB)
            nc.sync.dma_start(o_all[:, :], out_sb[:])

    nc.compile()
    return nc


def get_nc(v1=V1, v2c=V2CE, repeat=1, jd=None):
    key = (v1, v2c, repeat, jd)
    if key not in _BUILT:
        _BUILT[key] = _build_v2(v1, v2c, repeat, jd=jd)
    return _BUILT[key]


def _split16(x):
    hi = x.astype(np.float16)
    lo = (x - hi.astype(np.float32)).astype(np.float16)
    return hi, lo


def make_aug(gt, xp):
    """K=13 fp16 hi/lo-split matmul operand [A | G]; one matmul yields the
    near-exact squared-distance expansion |xp|^2 + |gt|^2 - 2 xp.gt."""
    v2c = xp.shape[0]
    v1 = gt.shape[0]
    a = (-2.0 * xp.T).astype(np.float32)  # (3, v2c)
    ah, al = _split16(a)
    nph, npl = _split16((xp * xp).sum(-1).astype(np.float32))
    g = gt.T.astype(np.float32)  # (3, v1)
    gh, gl = _split16(g)
    ngh, ngl = _split16((gt * gt).sum(-1).astype(np.float32))

    ag = np.zeros((KROWS, v2c + v1), np.float16)
    A = ag[:, :v2c]
    G = ag[:, v2c:]
    A[0:3] = ah
    A[3:6] = ah
    A[6:9] = al
    A[9] = nph
    A[10] = npl
    A[11] = 1.0
    A[12] = 1.0
    G[0:3] = gh
    G[3:6] = gl
    G[6:9] = gh
    G[9] = 1.0
    G[10] = 1.0
    G[11] = ngh
    G[12] = ngl
    return ag


def _compact_core(x_gt_b, x_pred_s, m_s, cf_s, v2c_eff):
    """Compact one core's active preds to the front, pad with origin points
    (the masked-pred equivalent) or a real pred if nothing is masked."""
    idx = np.flatnonzero(m_s > 0.0)
    n = len(idx)
    if n > v2c_eff:
        return None  # caller retries with a bigger variant
    xpc = np.zeros((v2c_eff, 3), np.float32)
    xpc[:n] = x_pred_s[idx]
    cfc = np.ones(v2c_eff, np.float32)
    cfc[:n] = cf_s[idx]
    if n == len(m_s) and n < v2c_eff:
        # no masked preds in this slice: origin may not be a valid pred,
        # pad with a duplicate of the first real pred instead
        xpc[n:] = xpc[0]
    return idx, xpc, cfc


def make_in_maps(x_gt, x_pred, mask, confidence, v2c_eff=V2CE):
    """Shard + compact full inputs into per-core input maps."""
    npt = v2c_eff // 128
    in_maps = []
    for c in range(N_CORES):
        b, s = divmod(c, SLICES)
        sl = slice(s * V2C, (s + 1) * V2C)
        comp = _compact_core(
            x_gt[b], x_pred[b, sl], mask[b, sl], confidence[b, sl], v2c_eff
        )
        assert comp is not None, "active pred overflow; use bigger v2c_eff"
        idx, xpc, cfc = comp
        ag = make_aug(x_gt[b], xpc)
        mc = cfc.reshape(npt, 128).T.copy()
        in_maps.append({"ag": ag, "mc": mc})
    return in_maps


def assemble_outputs(results, mask, v2c_eff=V2CE):
    """Gather per-core outputs back to full shapes (scatter compacted)."""
    npt = v2c_eff // 128
    loss_conf = np.zeros((B, V2), dtype=np.float32)
    loss_p2g = np.zeros((B, V2), dtype=np.float32)
    loss_g2p = np.full((B, V1), np.inf, dtype=np.float32)
    for c in range(N_CORES):
        b, s = divmod(c, SLICES)
        o = results[c]["o_all"]
        idx = np.flatnonzero(mask[b, s * V2C : (s + 1) * V2C] > 0.0)
        n = len(idx)
        pos = s * V2C + idx
        loss_conf[b, pos] = o[:, 0:npt].T.reshape(v2c_eff)[:n]
        loss_p2g[b, pos] = o[:, npt : 2 * npt].T.reshape(v2c_eff)[:n]
        part = results[c]["o_g2p"].reshape(V1)
        np.minimum(loss_g2p[b], part, out=loss_g2p[b])
    return loss_conf, loss_p2g, loss_g2p


def kernel(x_gt, x_pred, mask, confidence):
    from concourse.bass_utils import run_bass_kernel_spmd

    x_gt = np.asarray(x_gt)
    x_pred = np.asarray(x_pred)
    mask = np.asarray(mask)
    confidence = np.asarray(confidence)

    v2c_eff = V2CE
    maxn = max(
        int((mask[b, s * V2C : (s + 1) * V2C] > 0).sum())
        for b in range(B)
        for s in range(SLICES)
    )
    if maxn > v2c_eff:
        v2c_eff = -(-maxn // 128) * 128

    nc = get_nc(v2c=v2c_eff)
    in_maps = make_in_maps(x_gt, x_pred, mask, confidence, v2c_eff)
    res = run_bass_kernel_spmd(nc, in_maps, list(range(N_CORES)))
    return assemble_outputs(res.results, mask, v2c_eff)


# revision 13
# speedup vs baseline: 4.5594x; 1.1187x over previous
"""Bidirectional chamfer loss kernel for Trainium2 (8 NeuronCores).

Problem (hardcoded): B=2 batches, V1=8192 gt points, V2=8192 pred points, 3D.
  d2[b,i,j] = max(0, |xp_i|^2 + |gt_j|^2 - 2 xp_i.gt_j),  xp = x_pred * mask
  loss_pred2gt[b,i] = sqrt(min_j d2) * 100
  loss_gt2pred[b,j] = sqrt(min_i d2) * 100
  loss_conf = (loss_pred2gt * conf - ln(conf)) * mask ; loss_pred2gt *= mask

Sharding: 8 cores = 2 batches x 4 V2-slices (2048 preds/core vs full 8192 gt).
Each core computes its pred2gt slice exactly, and a partial gt2pred
(min over its preds); the host combines partials with np.minimum.

Mask compaction (host): masked preds all collapse to the origin and their
pred2gt outputs are zeroed by the reference, so each core only processes
its ACTIVE preds, compacted to the front and padded to V2CE=1792 slots
(14 tiles of 128; ~1638 active expected from an 80% mask). Pad slots are
origin points — exactly what masked preds become — so the gt2pred fold
stays correct; their pred2gt outputs are discarded on the host. If a
slice somehow has more active preds than V2CE, a larger variant is
compiled on the fly.

Device kernel V2 (per core, SPMD):
  - PE: ONE K=13 fp16 matmul per (pred-tile 128, gt-chunk 512). The fp16
    hi/lo split of the distance expansion is K-packed into a single
    matmul (PE cost depends only on the moving free dim, not K):
      rows 0-2:  a_hi(xyz)   x g_hi(xyz)
      rows 3-5:  a_hi(xyz)   x g_lo(xyz)
      rows 6-8:  a_lo(xyz)   x g_hi(xyz)
      rows 9-10: |xp|^2 hi/lo x 1
      rows 11-12: 1           x |gt|^2 hi/lo
    (a = -2*xp). fp16 products are exact in fp32 PSUM; the dropped
    lo*lo term is ~2^-22 relative -> d2 is near-exact.
  - Scalar: converts each PSUM tile [128,2048] to NEGATED fp16 in SBUF
    (activation Copy, scale=-1). Negation turns all min-reductions into
    max-reductions so the gt2pred partition reduce can use gpsimd
    partition_all_reduce (which supports max but not min).
  - DVE: per pred-tile row max-tree over the negated fp16 data (fp16
    tensor_tensor runs in 2x_1p mode), plus a column max-fold over the
    first JD gt columns.
  - GpSimd (Pool): column max-fold over the remaining gt columns during
    the main loop, then partition_all_reduce(max) over both column
    accumulators as the tail.
  - Epilogue: clamp, sqrt(-10000*x) == 100*sqrt(d2), conf math.

Outputs per core: o_all [128, 2*npt] = [conf_loss | p2g_loss] in the
(npt,128)->tile-major layout, and o_g2p [1, v1] = gt2pred partials.
"""

import numpy as np

B = 2
V1 = 8192  # gt points
V2 = 8192  # pred points (total)
N_CORES = 8
SLICES = N_CORES // B  # V2-slices per batch
V2C = V2 // SLICES  # pred points per core (before compaction)
V2CE = 1792  # compacted+padded pred slots per core

KROWS = 13

_BUILT = {}


def _jd(v1):
    """DVE share of the gt columns for the column fold (rest on gpsimd)."""
    return max(512, (v1 * 9 // 16) // 512 * 512)


def _build_v2(v1, v2c, repeat=1, jd=None):
    import concourse.tile as tile
    from concourse import bacc, bass_isa, mybir

    f32 = mybir.dt.float32
    f16 = mybir.dt.float16
    MAX = mybir.AluOpType.max
    MUL = mybir.AluOpType.mult
    SUB = mybir.AluOpType.subtract
    X = mybir.AxisListType.X
    AF = mybir.ActivationFunctionType

    npt = v2c // 128  # pred tiles
    W = min(2048, v1)  # PSUM super-tile width (4 banks)
    ng = v1 // W  # super-tiles per pred tile
    S = v2c + v1

    nc = bacc.Bacc()
    ag_in = nc.dram_tensor("ag", [KROWS, S], f16, kind="ExternalInput")
    mc_in = nc.dram_tensor("mc", [128, npt], f32, kind="ExternalInput")
    o_all = nc.dram_tensor("o_all", [128, 2 * npt], f32, kind="ExternalOutput")
    o_g2p = nc.dram_tensor("o_g2p", [1, v1], f32, kind="ExternalOutput")

    with tile.TileContext(nc) as tc:
        with (
            tc.tile_pool(name="persist", bufs=1) as P,
            tc.tile_pool(name="s16p", bufs=3) as S16P,
            tc.tile_pool(name="rowp", bufs=2) as RP,
            tc.tile_pool(name="small", bufs=1) as SP,
            tc.tile_pool(name="mmps", bufs=2, space="PSUM") as MMPS,
        ):
            AG = P.tile([KROWS, S], f16, tag="AG")
            A = AG[:, 0:v2c]
            G = AG[:, v2c:S]
            MC = P.tile([128, npt], f32, tag="MC")
            conf_ep = P.tile([128, npt], f32, tag="mc_sb")
            colD = P.tile([128, v1], f16, tag="colD")
            rowbuf = P.tile([128, npt * 512], f16, tag="rowbuf")
            p2g_neg = P.tile([128, npt], f32, tag="p2gneg")

            nc.sync.dma_start(AG[:], ag_in[:, :])
            nc.sync.dma_start(MC[:], mc_in[:, :])
            nc.vector.tensor_copy(conf_ep[:], MC[:])

            # ---- main loop ----
            for rep in range(repeat):
                for pt in range(npt):
                    s16 = S16P.tile([128, v1], f16, tag="s16")
                    psl = slice(pt * 128, (pt + 1) * 128)
                    for g in range(ng):
                        ps = MMPS.tile([128, W], f32, tag="mm")
                        for i in range(W // 512):
                            c0 = g * W + i * 512
                            nc.tensor.matmul(
                                ps[:, i * 512 : (i + 1) * 512],
                                A[:, psl],
                                G[:, c0 : c0 + 512],
                                start=True,
                                stop=True,
                            )
                        # PSUM f32 -> negated fp16 SBUF
                        nc.scalar.activation(
                            s16[:, g * W : (g + 1) * W], ps[:], AF.Copy, scale=-1.0
                        )
                    # row max-tree (negated data): fp16 TT runs 2x_1p;
                    # last level lands in rowbuf for one batched reduce
                    w = v1
                    src = s16
                    lvl = 0
                    while w > 1024:
                        h = w // 2
                        dst = RP.tile([128, h], f16, tag=f"t{lvl}")
                        nc.vector.tensor_tensor(
                            dst[:], src[:, 0:h], src[:, h:w], op=MAX
                        )
                        src, w, lvl = dst, h, lvl + 1
                    h = w // 2
                    nc.vector.tensor_tensor(
                        rowbuf[:, pt * 512 : pt * 512 + h],
                        src[:, 0:h],
                        src[:, h:w],
                        op=MAX,
                    )
                    # column fold (first pt of a pass initializes via a
                    # Scalar-engine copy -- Act has headroom, DVE is the wall)
                    if pt == 0:
                        nc.scalar.copy(colD[:], s16[:])
                    else:
                        nc.vector.tensor_tensor(
                            colD[:], colD[:], s16[:], op=MAX
                        )
                # one batched row reduce per pass
                nc.vector.tensor_reduce(
                    p2g_neg[:],
                    rowbuf[:].rearrange("p (t j) -> p t j", j=512),
                    axis=X,
                    op=MAX,
                )

            # ---- gt2pred tail ----
            g2p16 = SP.tile([128, v1], f16, tag="g2p16")
            nc.gpsimd.partition_all_reduce(
                g2p16[:], colD[:], 128, bass_isa.ReduceOp.max
            )
            g2prow = g2p16[0:1, :]
            nc.vector.tensor_scalar_min(g2prow, g2prow, 0.0)
            g2f = SP.tile([1, v1], f32, tag="g2f")
            # sqrt(-10000 * (-d2)) == 100*sqrt(d2)
            nc.scalar.activation(g2f[:], g2prow, AF.Sqrt, scale=-10000.0)
            nc.sync.dma_start(o_g2p[:, :], g2f[:])

            # ---- pred2gt epilogue (compacted preds are all active) ----
            out_sb = SP.tile([128, 2 * npt], f32, tag="out_sb")
            nc.vector.tensor_scalar_min(p2g_neg[:], p2g_neg[:], 0.0)
            ep = out_sb[:, npt : 2 * npt]
            nc.scalar.activation(ep, p2g_neg[:], AF.Sqrt, scale=-10000.0)
            lnc = SP.tile([128, npt], f32, tag="lnc")
            nc.scalar.activation(lnc[:], conf_ep[:], AF.Ln)
            o2 = SP.tile([128, npt], f32, tag="o2")
            nc.vector.tensor_tensor(o2[:], ep, conf_ep[:], op=MUL)
            nc.vector.tensor_tensor(out_sb[:, 0:npt], o2[:], lnc[:], op=SUB)
            nc.sync.dma_start(o_all[:, :], out_sb[:])

    nc.compile()
    return nc


def get_nc(v1=V1, v2c=V2CE, repeat=1, jd=None):
    key = (v1, v2c, repeat, jd)
    if key not in _BUILT:
        _BUILT[key] = _build_v2(v1, v2c, repeat, jd=jd)
    return _BUILT[key]


def _split16(x):
    hi = x.astype(np.float16)
    lo = (x - hi.astype(np.float32)).astype(np.float16)
    return hi, lo


def make_aug(gt, xp):
    """K=13 fp16 hi/lo-split matmul operand [A | G]; one matmul yields the
    near-exact squared-distance expansion |xp|^2 + |gt|^2 - 2 xp.gt."""
    v2c = xp.shape[0]
    v1 = gt.shape[0]
    a = (-2.0 * xp.T).astype(np.float32)  # (3, v2c)
    ah, al = _split16(a)
    nph, npl = _split16((xp * xp).sum(-1).astype(np.float32))
    g = gt.T.astype(np.float32)  # (3, v1)
    gh, gl = _split16(g)
    ngh, ngl = _split16((gt * gt).sum(-1).astype(np.float32))

    ag = np.zeros((KROWS, v2c + v1), np.float16)
    A = ag[:, :v2c]
    G = ag[:, v2c:]
    A[0:3] = ah
    A[3:6] = ah
    A[6:9] = al
    A[9] = nph
    A[10] = npl
    A[11] = 1.0
    A[12] = 1.0
    G[0:3] = gh
    G[3:6] = gl
    G[6:9] = gh
    G[9] = 1.0
    G[10] = 1.0
    G[11] = ngh
    G[12] = ngl
    return ag


def _compact_core(x_gt_b, x_pred_s, m_s, cf_s, v2c_eff):
    """Compact one core's active preds to the front, pad with origin points
    (the masked-pred equivalent) or a real pred if nothing is masked."""
    idx = np.flatnonzero(m_s > 0.0)
    n = len(idx)
    if n > v2c_eff:
        return None  # caller retries with a bigger variant
    xpc = np.zeros((v2c_eff, 3), np.float32)
    xpc[:n] = x_pred_s[idx]
    cfc = np.ones(v2c_eff, np.float32)
    cfc[:n] = cf_s[idx]
    if n == len(m_s) and n < v2c_eff:
        # no masked preds in this slice: origin may not be a valid pred,
        # pad with a duplicate of the first real pred instead
        xpc[n:] = xpc[0]
    return idx, xpc, cfc


def v2c_for(mask):
    """Smallest tile-rounded pred-slot count covering every core's slice."""
    maxn = max(
        int((np.asarray(mask)[b, s * V2C : (s + 1) * V2C] > 0).sum())
        for b in range(B)
        for s in range(SLICES)
    )
    return max(128, -(-maxn // 128) * 128)


def make_in_maps(x_gt, x_pred, mask, confidence, v2c_eff=None):
    """Shard + compact full inputs into per-core input maps."""
    if v2c_eff is None:
        v2c_eff = v2c_for(mask)
    npt = v2c_eff // 128
    in_maps = []
    for c in range(N_CORES):
        b, s = divmod(c, SLICES)
        sl = slice(s * V2C, (s + 1) * V2C)
        comp = _compact_core(
            x_gt[b], x_pred[b, sl], mask[b, sl], confidence[b, sl], v2c_eff
        )
        assert comp is not None, "active pred overflow; use bigger v2c_eff"
        idx, xpc, cfc = comp
        ag = make_aug(x_gt[b], xpc)
        mc = cfc.reshape(npt, 128).T.copy()
        in_maps.append({"ag": ag, "mc": mc})
    return in_maps


def assemble_outputs(results, mask, v2c_eff=V2CE):
    """Gather per-core outputs back to full shapes (scatter compacted)."""
    npt = v2c_eff // 128
    loss_conf = np.zeros((B, V2), dtype=np.float32)
    loss_p2g = np.zeros((B, V2), dtype=np.float32)
    loss_g2p = np.full((B, V1), np.inf, dtype=np.float32)
    for c in range(N_CORES):
        b, s = divmod(c, SLICES)
        o = results[c]["o_all"]
        idx = np.flatnonzero(mask[b, s * V2C : (s + 1) * V2C] > 0.0)
        n = len(idx)
        pos = s * V2C + idx
        loss_conf[b, pos] = o[:, 0:npt].T.reshape(v2c_eff)[:n]
        loss_p2g[b, pos] = o[:, npt : 2 * npt].T.reshape(v2c_eff)[:n]
        part = results[c]["o_g2p"].reshape(V1)
        np.minimum(loss_g2p[b], part, out=loss_g2p[b])
    return loss_conf, loss_p2g, loss_g2p


def kernel(x_gt, x_pred, mask, confidence):
    from concourse.bass_utils import run_bass_kernel_spmd

    x_gt = np.asarray(x_gt)
    x_pred = np.asarray(x_pred)
    mask = np.asarray(mask)
    confidence = np.asarray(confidence)

    v2c_eff = v2c_for(mask)
    nc = get_nc(v2c=v2c_eff)
    in_maps = make_in_maps(x_gt, x_pred, mask, confidence, v2c_eff)
    res = run_bass_kernel_spmd(nc, in_maps, list(range(N_CORES)))
    return assemble_outputs(res.results, mask, v2c_eff)
